# revision 13
# baseline (speedup 1.0000x reference)
"""Fused transformer block (LN -> QKV+RoPE -> attention -> out_proj) on 8
Trainium2 NeuronCores.

Sharding: batch (2-way) x heads (4-way) = 8 cores. Core c handles batch
b = c // 4 and the 4 heads starting at 4*(c%4). Each core produces the
out_proj partial sum over its 256 dh-dims; the host sums 4 partials per
batch and adds b_out.

Device math, per core (matmuls in fp32r: 1 PE col/cycle vs 4 for fp32;
every fp32r matmul input is produced as fp32r - DMA from an fp32r dram
tensor or an engine op with fp32r out - per the BIR verifier rule):
- x passed transposed: xT [D, S] (d on partitions, s free).
- all bulk weight loads issued up front so no weight DMA queues behind
  a stats-dependent bounce DMA (HWDGE rings drain FIFO per engine).
- LN stats via TensorE ones-matmuls; x^2 on ScalarE (Square, idle in
  the prefix); rstd via Rsqrt. mu kept negated: mean-centering is a
  K=1 matmul accumulation (lhsT = wsum row, rhs = -mu) folded into the
  QKV PSUM group, so no mu broadcast is needed.
- RoPE in [e, s] layout; DVE reads zq straight from PSUM for the cos
  product; the rotate-half source is a ScalarE PSUM->SBUF copy, swapped
  via partition-sliced SBUF->SBUF DMAs on the gpsimd queue; q also gets
  rstd (DVE); k's rstd is folded into exp's per-partition scale.
- attention runs per i-half (po [65, 1024] = 2 PSUM banks, pool bufs=2)
  so the per-head softmax-denominator normalization (reciprocal + DRAM
  bounce broadcast) fully overlaps the next head's matmuls/exp.
- scores^T[j,i] per (head, j-tile) with K=64; exp on ScalarE
  (scale = rstd_k[j]/8); o^T accumulated over j with lhsT = [v | 1]
  (M=65, row 64 = softmax denominators for free).
- out_proj partial from o^T tiles -> HBM; PSUM->SBUF copies alternate
  ScalarE/VectorE; host reduces + adds b_out.
"""
import sys
sys.path.insert(0, "/opt/trn_rl_repo")
import numpy as np
import ml_dtypes
BF16 = ml_dtypes.bfloat16

B, S, D = 2, 2048, 1024
HEADS, HDIM = 16, 64
HALF = HDIM // 2
ROPE_THETA = 10000.0
N_CORES = 8
HPC = HEADS // 4            # heads per core = 4
EC = HPC * HDIM             # per-core q (or k, or v) width = 256
P = 128
NK = D // P                 # 8 d-tiles
NS = S // P                 # 16 s-tiles
VW = HDIM + 1               # v block width incl. ones column = 65
SH = S // 2                 # i-half width = 1024

_cache = {}


def _build():
    import os
    import contextlib
    import concourse.bass as bass
    import concourse.bacc as bacc
    import concourse.tile as tile
    from concourse import mybir
    fp32 = mybir.dt.float32
    fp32r = mybir.dt.float32r
    bf16 = mybir.dt.bfloat16
    OP = mybir.AluOpType
    AF = mybir.ActivationFunctionType
    _abl = os.environ.get("ABLATE", "")

    nc = bacc.Bacc("TRN2", target_bir_lowering=False, debug=False,
                   enable_asserts=True, num_devices=N_CORES)

    xT = nc.dram_tensor("xT", [D, S], bf16, kind="ExternalInput").ap()
    wqkT = nc.dram_tensor("wqkT", [D, 2 * EC], bf16, kind="ExternalInput").ap()
    wvT = nc.dram_tensor("wvT", [D, EC], bf16, kind="ExternalInput").ap()
    woT = nc.dram_tensor("woT", [EC, D], bf16, kind="ExternalInput").ap()
    wsum_qk = nc.dram_tensor("wsum_qk", [2 * EC], bf16,
                             kind="ExternalInput").ap()
    wvsum = nc.dram_tensor("wvsum", [EC], fp32, kind="ExternalInput").ap()
    cosf = nc.dram_tensor("cosf", [P, S], fp32, kind="ExternalInput").ap()
    sinsg = nc.dram_tensor("sinsg", [P, S], fp32, kind="ExternalInput").ap()
    out = nc.dram_tensor("out", [S, D], bf16, kind="ExternalOutput").ap()

    wqk_r = wqkT.rearrange("(k p) e -> p k e", p=P)
    wv_r = wvT.rearrange("(k p) e -> p k e", p=P)
    wo_r = woT.rearrange("(k p) e -> p k e", p=P)

    reps = int(os.environ.get("KREPS", "1"))
    with tile.TileContext(nc) as tc:
     for _rep in range(reps):
      with tc.tile_pool(name=f"singles{_rep}", bufs=1) as singles, \
           tc.tile_pool(name=f"dram_scr{_rep}", bufs=1,
                        space="DRAM") as dram_scr:
        qk_sb = singles.tile([P, 4, S], bf16)             # 32KB/part
        v_sb = singles.tile([P, NS, HPC * VW], bf16)      # 16.25KB/part
        rstdT = singles.tile([P, NS], fp32)
        muT = singles.tile([P, NS], bf16)
        muTf = singles.tile([P, NS], fp32)
        rstdT8 = singles.tile([P, NS], fp32)
        onep = singles.tile([P, 2], fp32)
        nc.vector.memset(onep[:], 1.0)
        nc.vector.memset(onep[0:1, 1:2], 1e-5)
        ones_rt = singles.tile([P, 1], bf16)
        nc.vector.tensor_copy(out=ones_rt[:], in_=onep[:, 0:1])
        ones_sb = ones_rt[:]
        eps_sb = onep[0:1, 1:2]
        # ones column of [v | 1] (memset can't write fp32r)
        nc.vector.tensor_copy(
            out=v_sb[:].rearrange("p t (h w) -> p t h w", w=VW)[:, :, :,
                                                               HDIM:VW],
            in_=onep[:, 0:1].broadcast_to([P, NS, HPC, 1]))

        with tc.tile_pool(name="ph1a", bufs=1) as ph1a:
            # ---------------- bulk loads, consumption order ----------------
            xT_sb = ph1a.tile([P, NK, S], bf16)           # 64KB/part
            xT_r = xT.rearrange("(k p) s -> p k s", p=P)
            for k in range(NK):     # split across queues: 8 x 1MB
                eng = (nc.sync, nc.gpsimd, nc.scalar)[k % 3]
                eng.dma_start(out=xT_sb[:, k, :], in_=xT_r[:, k, :])
            wqk_sb = ph1a.tile([P, 4, NK, P], bf16)       # 16KB/part
            for e in range(4):
                nc.scalar.dma_start(out=wqk_sb[:, e, :, :],
                                    in_=wqk_r[:, :, e * P:(e + 1) * P])
            wv_sb = ph1a.tile([P, NK, EC], bf16)          # 8KB/part
            for k2 in range(2):
                nc.sync.dma_start(out=wv_sb[:, 4 * k2:4 * (k2 + 1), :],
                                  in_=wv_r[:, 4 * k2:4 * (k2 + 1), :])
            wsqk_row = ph1a.tile([1, 2 * EC], bf16)
            nc.sync.dma_start(out=wsqk_row[:], in_=wsum_qk.unsqueeze(0))
            wsv_b = ph1a.tile([P, EC], fp32)
            nc.sync.dma_start(
                out=wsv_b[:],
                in_=bass.AP(tensor=wvsum.tensor, offset=wvsum.offset,
                            ap=[[0, P], [1, EC]]))
            cos_sb = ph1a.tile([P, S], fp32)
            sin_sb = ph1a.tile([P, S], fp32)
            nc.gpsimd.dma_start(out=cos_sb[:], in_=cosf)
            nc.gpsimd.dma_start(out=sin_sb[:], in_=sinsg)
            rstd_b = ph1a.tile([P, S], fp32)
            mu_sb = ph1a.tile([1, S], bf16)           # holds -mu

            # ---------------- phase 0: LN stats ----------------
            with tc.tile_pool(name="p0ps_a", bufs=2, space="PSUM") as p0ps_a, \
                 tc.tile_pool(name="p0ps_b", bufs=1, space="PSUM") as p0ps_b, \
                 tc.tile_pool(name="p0scr", bufs=1) as p0scr, \
                 tc.tile_pool(name="p0tmp", bufs=2) as p0tmp:
                ssq_sb = p0scr.tile([1, S], fp32)
                rstd_sb = p0scr.tile([1, S], fp32)
                for c in range(4):
                    ps_sum = p0ps_a.tile([1, 512], fp32, tag="ps")
                    for k in range(NK):
                        nc.tensor.matmul(ps_sum[:], ones_sb,
                                         xT_sb[:, k, c * 512:(c + 1) * 512],
                                         start=(k == 0), stop=(k == NK - 1))
                    nc.scalar.mul(out=mu_sb[:, c * 512:(c + 1) * 512],
                                  in_=ps_sum[:], mul=-1.0 / D)
                psq = [p0ps_b.tile([1, 512], fp32, tag=f"psq{c}", name=f"psq{c}")
                       for c in range(4)]
                for k in range(NK):
                    for h2 in range(2):
                        xsq = p0tmp.tile([P, S // 2], bf16, tag="xsq")
                        nc.scalar.activation(
                            xsq[:], xT_sb[:, k, h2 * 1024:(h2 + 1) * 1024],
                            AF.Square)
                        for c in range(2):
                            ci = h2 * 2 + c
                            nc.tensor.matmul(psq[ci][:], ones_sb,
                                             xsq[:, c * 512:(c + 1) * 512],
                                             start=(k == 0), stop=(k == NK - 1),
                                             skip_group_check=True)
                for c in range(4):
                    nc.vector.tensor_copy(out=ssq_sb[:, c * 512:(c + 1) * 512],
                                          in_=psq[c][:])
                # var = ssq/D - mu^2 ; rstd = 1/sqrt(var + eps)
                nc.vector.tensor_mul(rstd_sb[:], mu_sb[:], mu_sb[:])
                nc.vector.scalar_tensor_tensor(out=rstd_sb[:], in0=ssq_sb[:],
                                               scalar=1.0 / D, in1=rstd_sb[:],
                                               op0=OP.mult, op1=OP.subtract)
                nc.scalar.activation(rstd_sb[:], rstd_sb[:], AF.Sqrt,
                                     bias=eps_sb)
                nc.vector.reciprocal(out=rstd_sb[:], in_=rstd_sb[:])

                # SBUF->SBUF partition-broadcast is illegal; bounce via DRAM.
                mu_d = dram_scr.tile([1, S], bf16)
                rstd_d = dram_scr.tile([1, S], fp32)
                nc.sync.dma_start(out=mu_d[:], in_=mu_sb[:])
                nc.sync.dma_start(out=rstd_d[:], in_=rstd_sb[:])
                _md, _rd = mu_d[:], rstd_d[:]
                for hh in range(2):
                    o0 = hh * SH
                    nc.sync.dma_start(
                        out=rstd_b[:, o0:o0 + SH],
                        in_=bass.AP(tensor=_rd.tensor,
                                    offset=_rd.offset + o0,
                                    ap=[[0, P], [1, SH]]))
                # transposed per-s-tile scalars: [p, t] = vec[t*128 + p]
                nc.sync.dma_start(
                    out=rstdT[:],
                    in_=bass.AP(tensor=_rd.tensor, offset=_rd.offset,
                                ap=[[1, P], [P, NS]]))
                nc.sync.dma_start(
                    out=muT[:],
                    in_=bass.AP(tensor=_md.tensor, offset=_md.offset,
                                ap=[[1, P], [P, NS]]))
                nc.vector.tensor_scalar_mul(rstdT8[:], rstdT[:],
                                            float(HDIM) ** -0.5)
                nc.vector.tensor_copy(out=muTf[:], in_=muT[:])

            # ---------------- phase 1a: Q/K matmuls + RoPE ----------------
            with tc.tile_pool(name="p1psum", bufs=3, space="PSUM") as p1psum, \
                 tc.tile_pool(name="p1tmp", bufs=2) as p1tmp:
                for e in range(4):
                    for sh in range(2):
                        s0 = sh * SH
                        zq = p1psum.tile([P, SH], fp32, tag="zqk")
                        for c in range(2):
                            c0 = c * 512
                            for k in range(NK):
                                nc.tensor.matmul(
                                    zq[:, c0:c0 + 512],
                                    wqk_sb[:, e, k, :],
                                    xT_sb[:, k, s0 + c0:s0 + c0 + 512],
                                    start=(k == 0), stop=False)
                            # mean-centering: zq += wsum_e (x) (-mu)
                            nc.tensor.matmul(
                                zq[:, c0:c0 + 512],
                                wsqk_row[0:1, e * P:(e + 1) * P],
                                mu_sb[0:1, s0 + c0:s0 + c0 + 512],
                                start=False, stop=True)
                        # y = zq * sin2 (sign pre-arranged so the rotate-
                        # half swap happens AFTER the multiply: no PSUM copy)
                        y = p1tmp.tile([P, SH], fp32, tag="y")
                        nc.vector.tensor_mul(y[:], zq[:],
                                             sin_sb[:, s0:s0 + SH])
                        ysw = p1tmp.tile([P, SH], fp32, tag="ysw")
                        for g in range(2):
                            b0 = g * HDIM
                            nc.gpsimd.dma_start(out=ysw[b0:b0 + HALF, :],
                                                in_=y[b0 + HALF:b0 + HDIM, :])
                            nc.gpsimd.dma_start(out=ysw[b0 + HALF:b0 + HDIM, :],
                                                in_=y[b0:b0 + HALF, :])
                        t1 = p1tmp.tile([P, SH], fp32, tag="t1")
                        nc.vector.tensor_mul(t1[:], zq[:],
                                             cos_sb[:, s0:s0 + SH])
                        if e < 2:   # q side: multiply by rstd as well
                            nc.vector.tensor_add(t1[:], t1[:], ysw[:])
                            nc.vector.tensor_mul(qk_sb[:, e, s0:s0 + SH],
                                                 t1[:], rstd_b[:, s0:s0 + SH])
                        else:
                            nc.vector.tensor_add(qk_sb[:, e, s0:s0 + SH],
                                                 t1[:], ysw[:])

            # ---------------- phase 1b: V (natural layout) ----------------
            with tc.tile_pool(name="p1vps", bufs=3, space="PSUM") as p1vps, \
                 tc.tile_pool(name="p1vt", bufs=3) as p1vt:
                for t in range(NS):
                    zv = p1vps.tile([P, EC], fp32, tag="zv")
                    for k in range(NK):
                        nc.tensor.matmul(zv[:], xT_sb[:, k, t * P:(t + 1) * P],
                                         wv_sb[:, k, :],
                                         start=(k == 0), stop=(k == NK - 1))
                    # t2v = wvsum * (-mu)_s * rstd_s
                    t2v = p1vt.tile([P, EC], fp32, tag="t2v")
                    nc.vector.tensor_scalar(out=t2v[:], in0=wsv_b[:],
                                            scalar1=muTf[:, t:t + 1],
                                            scalar2=rstdT[:, t:t + 1],
                                            op0=OP.mult, op1=OP.mult)
                    # v = rstd_s * Zv + t2v
                    nc.vector.scalar_tensor_tensor(
                        out=v_sb[:, t, :].rearrange("p (h w) -> p h w",
                                                    h=HPC)[:, :, 0:HDIM],
                        in0=zv[:].rearrange("p (h d) -> p h d", h=HPC),
                        scalar=rstdT[:, t:t + 1],
                        in1=t2v[:].rearrange("p (h d) -> p h d", h=HPC),
                        op0=OP.mult, op1=OP.add)

        # ---------------- phase 2: attention (per i-half) ----------------
        if _abl == "p01":
            continue
        with tc.tile_pool(name="late", bufs=1) as late:
          oT_sb = late.tile([P, 2, S], bf16)      # o^T (4 heads x 64 rows)
          wo_sb = late.tile([P, 2, D], bf16)
          for k in range(2):
              nc.sync.dma_start(out=wo_sb[:, k, :], in_=wo_r[:, k, :])
          with tc.tile_pool(name="ps_s", bufs=2, space="PSUM") as ps_s, \
               tc.tile_pool(name="ps_o", bufs=2, space="PSUM") as ps_o, \
               tc.tile_pool(name="p2tmp", bufs=3) as p2tmp, \
               tc.tile_pool(name="p2rec", bufs=2) as p2rec, \
               tc.tile_pool(name="p2recd", bufs=2, space="DRAM") as p2recd:
            for ih in range(2):
                i0 = ih * SH
                for h in range(HPC):
                    et = h // 2
                    ep = (h % 2) * HDIM
                    po = ps_o.tile([VW, SH], fp32, tag="po")
                    for j in range(NS):
                        pscore = ps_s.tile([P, SH], fp32, tag="ps")
                        for c in range(2):
                            c0 = c * 512
                            nc.tensor.matmul(
                                pscore[:, c0:c0 + 512],
                                qk_sb[ep:ep + HDIM, 2 + et,
                                      j * P:(j + 1) * P],
                                qk_sb[ep:ep + HDIM, et,
                                      i0 + c0:i0 + c0 + 512],
                                start=True, stop=True)
                        p_sb = p2tmp.tile([P, SH], bf16, tag="p")
                        nc.scalar.activation(p_sb[:], pscore[:], AF.Exp,
                                             scale=rstdT8[:, j:j + 1])
                        for c in range(2):
                            nc.tensor.matmul(po[:, c * 512:(c + 1) * 512],
                                             v_sb[:, j, h * VW:(h + 1) * VW],
                                             p_sb[:, c * 512:(c + 1) * 512],
                                             start=(j == 0), stop=(j == NS - 1),
                                             skip_group_check=True)
                    rec = p2rec.tile([1, SH], fp32, tag="rec")
                    nc.vector.reciprocal(out=rec[:], in_=po[HDIM:HDIM + 1, :])
                    rec_d = p2recd.tile([1, SH], fp32, tag="recd", name="rec_d")
                    nc.sync.dma_start(out=rec_d[:], in_=rec[:])
                    recb = p2rec.tile([HDIM, SH], fp32, tag="recb")
                    _rc = rec_d[:]
                    nc.sync.dma_start(
                        out=recb[:],
                        in_=bass.AP(tensor=_rc.tensor, offset=_rc.offset,
                                    ap=[[0, HDIM], [1, SH]]))
                    nc.vector.tensor_mul(oT_sb[ep:ep + HDIM, et, i0:i0 + SH],
                                         po[0:HDIM, :], recb[:])

          # ---------------- phase 3: out_proj partial ----------------
          with tc.tile_pool(name="p3psum", bufs=2, space="PSUM") as p3psum, \
               tc.tile_pool(name="p3tmp", bufs=3) as p3tmp:
                for t in range(NS):
                    pout = p3psum.tile([P, D], fp32, tag="pout")
                    for c in range(2):
                        for k in range(2):
                            nc.tensor.matmul(pout[:, c * 512:(c + 1) * 512],
                                             oT_sb[:, k, t * P:(t + 1) * P],
                                             wo_sb[:, k, c * 512:(c + 1) * 512],
                                             start=(k == 0), stop=(k == 1))
                    ot = p3tmp.tile([P, D], bf16, tag="ot")
                    if t % 2 == 0:
                        nc.vector.tensor_copy(out=ot[:], in_=pout[:])
                    else:
                        nc.scalar.copy(out=ot[:], in_=pout[:])
                    eng = nc.sync if t % 2 == 0 else nc.gpsimd
                    eng.dma_start(out=out[t * P:(t + 1) * P, :], in_=ot[:])

    nc.compile()
    return nc


def _host_inputs(x, ln_g, ln_b, w_qkv, w_out):
    wq = w_qkv[0:D] * ln_g[None, :]
    wk = w_qkv[D:2 * D] * ln_g[None, :]
    wv = w_qkv[2 * D:3 * D] * ln_g[None, :]
    if np.abs(w_qkv.astype(np.float32) @ ln_b.astype(np.float32)).max() != 0.0:
        raise NotImplementedError("nonzero ln_b not supported")
    inv = 1.0 / (ROPE_THETA ** (np.arange(0, HALF, dtype=np.float32) / HALF))
    fr = np.arange(S, dtype=np.float32)[:, None] * inv[None, :]
    cos = np.cos(fr).T.astype(np.float32)          # [32, S]
    sin = np.sin(fr).T.astype(np.float32)
    # row layout per 64-group: [lo(32); hi(32)]; cos same both halves.
    cosf = np.tile(cos, (4, 1))                    # [128, S]
    # rot_lo = lo*c - hi*s ; rot_hi = hi*c + lo*s. y = zq*sin2 is computed
    # BEFORE the rotate-half swap, so sin2 rows are [+s (lo rows, feeds the
    # hi output after the swap); -s (hi rows, feeds the lo output)].
    sinsg = np.tile(np.concatenate([sin, -sin], 0), (2, 1))
    ins = []
    for core in range(N_CORES):
        b = core // 4
        h0 = (core % 4) * HPC
        sl = slice(h0 * HDIM, (h0 + HPC) * HDIM)
        wq_c, wk_c, wv_c = wq[sl], wk[sl], wv[sl]
        qk = np.concatenate([wq_c, wk_c], 0)
        ins.append({
            "xT": np.ascontiguousarray(x[b].T.astype(BF16)),
            "wqkT": np.ascontiguousarray(qk.T.astype(BF16)),
            "wvT": np.ascontiguousarray(wv_c.T.astype(BF16)),
            "woT": np.ascontiguousarray(w_out[:, sl].T.astype(BF16)),
            "wsum_qk": qk.sum(1).astype(BF16),
            "wvsum": wv_c.sum(1).astype(np.float32),
            "cosf": cosf, "sinsg": sinsg,
        })
    return ins


def kernel(x, ln_g, ln_b, w_qkv, w_out, b_out):
    from concourse import bass_utils
    x = np.asarray(x, np.float32)
    ln_g = np.asarray(ln_g, np.float32)
    ln_b = np.asarray(ln_b, np.float32)
    w_qkv = np.asarray(w_qkv, np.float32)
    w_out = np.asarray(w_out, np.float32)
    b_out = np.asarray(b_out, np.float32)
    if "nc" not in _cache:
        _cache["nc"] = _build()
    ins = _host_inputs(x, ln_g, ln_b, w_qkv, w_out)
    res = bass_utils.run_bass_kernel_spmd(_cache["nc"], ins,
                                          core_ids=list(range(N_CORES)))
    _cache["last_results"] = res
    out = np.zeros((B, S, D), np.float32)
    for core in range(N_CORES):
        out[core // 4] += np.asarray(res.results[core]["out"],
                                     dtype=np.float32)
    out += b_out[None, None, :]
    return out
